# revision 8
# baseline (speedup 1.0000x reference)
"""DeepSeekV3 MLA Bass kernel for 8 Trainium2 NeuronCores.

Sharding: tensor-parallel across heads x data-parallel across batch.
core c: batch b = c//4, head group g = c%4 (heads 4g..4g+3).
Each core computes the shared latent path (replicated within a batch group),
attention for its 4 heads, and a partial o-projection; the host sums the 4
partial outputs per batch (TP all-reduce done host-side at gather time).

All activations live feature-major ([feature, token]) on chip so every matmul
has its contraction dim on partitions with zero on-device transposes:
  - lat = x @ [Wqa|Wkva]        -> psum [f, t], lhsT = W chunk, rhs = x^T chunk
  - scores s^T[kt, qt]          -> lhsT = k_fm block, rhs = q_fm
  - av^T[dv, qt] = v^T e^T      -> lhsT = v token-major block, rhs = e^T
  - out[t, dout]                -> lhsT = av_fm block, rhs = Wo slice
softmax denominator = ones-vector matmul over e^T (partition reduce on PE).
rmsnorm r1 of the outer norm cancels against the q/kv rmsnorms (up to eps,
~1e-6 relative); it survives only in k_rot, where it is applied explicitly
from a sum-of-squares computed via the same ones-matmul trick.
"""

import math
from contextlib import ExitStack

import numpy as np
import ml_dtypes

import concourse.bacc as bacc
import concourse.tile as tile
from concourse import mybir
from concourse import bass_utils

AF = mybir.ActivationFunctionType
ALU = mybir.AluOpType
BF16 = mybir.dt.bfloat16
F32 = mybir.dt.float32

D = 2048
NH = 16
DR = 64
DN = 128
DV = 128
RQ = 128
RKV = 128
DQK = DR + DN
B, S = 2, 2048
EPS = 1e-6
NCORES = 8
HPC = 4            # heads per core
NJ = S // 512      # 4 token tiles of 512
KCH = D // 128     # 16 contraction chunks over D

_CACHE = {}


def _rsqrt(nc, stats, out_bcast, ss_psum, mean_scale, eps_ap):
    """out_bcast[128,512] = 1/sqrt(ss*mean_scale + EPS) per token (Newton),
    replicated to all partitions via gpsimd."""
    m = stats.tile([1, 512], F32, tag="st_m")
    nc.scalar.activation(m[:], ss_psum, AF.Identity, bias=eps_ap,
                         scale=float(mean_scale))
    s = stats.tile([1, 512], F32, tag="st_s")
    nc.scalar.activation(s[:], m[:], AF.Sqrt)
    r0 = stats.tile([1, 512], F32, tag="st_r0")
    nc.vector.reciprocal(r0[:], s[:])
    t = stats.tile([1, 512], F32, tag="st_t")
    nc.vector.tensor_mul(t[:], r0[:], r0[:])
    nc.vector.tensor_mul(t[:], t[:], m[:])
    nc.vector.tensor_scalar(t[:], t[:], -0.5, 1.5, op0=ALU.mult, op1=ALU.add)
    r = stats.tile([1, 512], F32, tag="st_r")
    nc.vector.tensor_mul(r[:], r0[:], t[:])
    nc.gpsimd.partition_broadcast(out_bcast, r[:])


def _build():
    if "nc" in _CACHE:
        return _CACHE["nc"]

    nc = bacc.Bacc("TRN2", target_bir_lowering=False, debug=False,
                   num_devices=NCORES)

    # ---- I/O ----
    xT = nc.dram_tensor("xT", [128, KCH, S], BF16, kind="ExternalInput").ap()
    wlat = nc.dram_tensor("wlat", [128, KCH, 384], BF16, kind="ExternalInput").ap()
    wqb = nc.dram_tensor("wqb", [128, 4, 128], BF16, kind="ExternalInput").ap()
    wqbr = nc.dram_tensor("wqbr", [128, 4, 128], BF16, kind="ExternalInput").ap()
    wk = nc.dram_tensor("wk", [128, HPC, 128], BF16, kind="ExternalInput").ap()
    wv = nc.dram_tensor("wv", [128, 512], BF16, kind="ExternalInput").ap()
    wo = nc.dram_tensor("wo", [128, HPC, D], BF16, kind="ExternalInput").ap()
    cos2 = nc.dram_tensor("cos2", [128, S], BF16, kind="ExternalInput").ap()
    sin2 = nc.dram_tensor("sin2", [128, S], BF16, kind="ExternalInput").ap()
    cos1 = nc.dram_tensor("cos1", [64, S], F32, kind="ExternalInput").ap()
    sin1 = nc.dram_tensor("sin1", [64, S], F32, kind="ExternalInput").ap()
    tril = nc.dram_tensor("tril", [128, 128], BF16, kind="ExternalInput").ap()
    kvw = nc.dram_tensor("kvw", [128, 1], F32, kind="ExternalInput").ap()

    out_p = nc.dram_tensor("out_p", [S, D], F32, kind="ExternalOutput").ap()
    kvp_T = nc.dram_tensor("kvp_T", [RKV, S], F32, kind="ExternalOutput").ap()
    krot_T = nc.dram_tensor("krot_T", [DR, S], F32, kind="ExternalOutput").ap()

    with tile.TileContext(nc) as tc, ExitStack() as ctx:
        consts = ctx.enter_context(tc.tile_pool(name="consts", bufs=1))
        persist = ctx.enter_context(tc.tile_pool(name="persist", bufs=1))

        sb_wqb = consts.tile([128, 4, 128], BF16)
        sb_wqbr = consts.tile([128, 4, 128], BF16)
        sb_wk = consts.tile([128, HPC, 128], BF16)
        sb_wv = consts.tile([128, 512], BF16)
        sb_wo = consts.tile([128, HPC, D], BF16)
        sb_cos2 = consts.tile([128, S], BF16)
        sb_sin2 = consts.tile([128, S], BF16)
        sb_tril = consts.tile([128, 128], BF16)
        sb_kvw = consts.tile([128, 1], F32)
        sb_ones = consts.tile([128, 1], BF16)
        sb_eps = consts.tile([128, 1], F32)

        nc.sync.dma_start(out=sb_wqb[:], in_=wqb)
        nc.sync.dma_start(out=sb_wqbr[:], in_=wqbr)
        nc.sync.dma_start(out=sb_wk[:], in_=wk)
        nc.sync.dma_start(out=sb_wv[:], in_=wv)
        nc.sync.dma_start(out=sb_wo[:], in_=wo)
        nc.sync.dma_start(out=sb_cos2[:], in_=cos2)
        nc.sync.dma_start(out=sb_sin2[:], in_=sin2)
        nc.sync.dma_start(out=sb_tril[:], in_=tril)
        nc.sync.dma_start(out=sb_kvw[:], in_=kvw)
        nc.vector.memset(sb_ones[:], 1.0)
        nc.vector.memset(sb_eps[:], EPS)

        qa_n = persist.tile([128, S], BF16)
        kv_n = persist.tile([128, S], BF16)
        q_nope = persist.tile([128, HPC, S], BF16)
        q_rope = persist.tile([128, 2, S], BF16)  # [h_even|h_odd] x pair
        krot2 = persist.tile([128, S], BF16)      # krot stacked twice
        k_nope = persist.tile([128, HPC, S], BF16)
        v_tok = persist.tile([128, S // 128, 512], BF16)

        # ---------------- Phase A+B: latent path + projections ----------
        with ExitStack() as actx:
            apool = actx.enter_context(tc.tile_pool(name="apool", bufs=1))
            xjp = actx.enter_context(tc.tile_pool(name="xjp", bufs=2))
            sqp = actx.enter_context(tc.tile_pool(name="sqp", bufs=2))
            stats = actx.enter_context(tc.tile_pool(name="stats", bufs=1))
            rpool = actx.enter_context(tc.tile_pool(name="rpool", bufs=2))
            ropep = actx.enter_context(tc.tile_pool(name="ropep", bufs=2))
            stgp = actx.enter_context(tc.tile_pool(name="stgp", bufs=2))
            trigp = actx.enter_context(tc.tile_pool(name="trigp", bufs=2))
            ps_lat = actx.enter_context(
                tc.tile_pool(name="ps_lat", bufs=3, space="PSUM"))
            ps_ssx = actx.enter_context(
                tc.tile_pool(name="ps_ssx", bufs=1, space="PSUM"))
            ps_ssl = actx.enter_context(
                tc.tile_pool(name="ps_ssl", bufs=2, space="PSUM"))
            ps_proj = actx.enter_context(
                tc.tile_pool(name="ps_proj", bufs=2, space="PSUM"))

            sb_wlat = apool.tile([128, KCH, 384], BF16)
            nc.sync.dma_start(out=sb_wlat[:], in_=wlat)

            for j in range(NJ):
                tt = slice(512 * j, 512 * j + 512)
                xj = xjp.tile([128, KCH, 512], BF16, tag="xj")
                nc.sync.dma_start(out=xj[:], in_=xT[:, :, tt])
                c1 = trigp.tile([64, 512], F32, tag="c1")
                s1 = trigp.tile([64, 512], F32, tag="s1")
                nc.sync.dma_start(out=c1[:], in_=cos1[:, tt])
                nc.sync.dma_start(out=s1[:], in_=sin1[:, tt])

                # sum of squares of x per token -> r1 (used only by k_rot)
                ssx = ps_ssx.tile([1, 512], F32, tag="ssx")
                for k in range(KCH):
                    xsq = sqp.tile([128, 512], BF16, tag="xsq")
                    nc.vector.tensor_mul(xsq[:], xj[:, k], xj[:, k])
                    nc.tensor.matmul(ssx[:], sb_ones[:], xsq[:],
                                     start=(k == 0), stop=(k == KCH - 1))
                r1 = rpool.tile([128, 512], F32, tag="r1")
                _rsqrt(nc, stats, r1[:], ssx[:], 1.0 / D, sb_eps[0:1, :])

                # latent projections: q_a 0:128, kv 128:256,
                # rope 256:320, rope_rot 320:384 (host-permuted copy so the
                # rotate-half needs no cross-partition DVE ops)
                lp_rope = None
                for fc, (f0, fp) in enumerate([(0, 128), (128, 128),
                                               (256, 64), (320, 64)]):
                    lp = ps_lat.tile([128, 512], F32, tag="lat")
                    for k in range(KCH):
                        nc.tensor.matmul(lp[:fp], sb_wlat[:, k, f0:f0 + fp],
                                         xj[:, k], start=(k == 0),
                                         stop=(k == KCH - 1))
                    if fc < 2:
                        # per-token rmsnorm over the 128 latent features
                        lsq = sqp.tile([128, 512], BF16, tag="lsq")
                        nc.scalar.activation(lsq[:], lp[:], AF.Square)
                        ssl = ps_ssl.tile([1, 512], F32, tag="ssl")
                        nc.tensor.matmul(ssl[:], sb_ones[:], lsq[:],
                                         start=True, stop=True)
                        rr = rpool.tile([128, 512], F32, tag="rr")
                        _rsqrt(nc, stats, rr[:], ssl[:], 1.0 / 128, sb_eps[0:1, :])
                        dst = qa_n if fc == 0 else kv_n
                        nc.vector.tensor_mul(dst[:, tt], lp[:], rr[:])
                        if fc == 1:
                            kvo = stgp.tile([128, 512], F32, tag="kvo")
                            nc.vector.scalar_tensor_tensor(
                                kvo[:], lp[:], sb_kvw[:], rr[:],
                                op0=ALU.mult, op1=ALU.mult)
                            nc.sync.dma_start(out=kvp_T[:, tt], in_=kvo[:])
                    elif fc == 2:
                        lp_rope = lp
                    else:
                        # krot = raw*cos + rot*sin, then * r1
                        kro = stgp.tile([64, 512], F32, tag="kro")
                        t1 = ropep.tile([64, 512], F32, tag="kr_t1")
                        u = ropep.tile([64, 512], F32, tag="kr_u")
                        nc.vector.tensor_mul(t1[:], lp_rope[0:64], c1[:])
                        nc.vector.tensor_mul(u[:], lp[0:64], s1[:])
                        nc.vector.tensor_add(kro[:], t1[:], u[:])
                        nc.vector.tensor_mul(kro[:], kro[:], r1[0:64])
                        nc.sync.dma_start(out=krot_T[:, tt], in_=kro[:])
                        nc.vector.tensor_copy(out=krot2[0:64, tt], in_=kro[:])
                        nc.sync.dma_start(out=krot2[64:128, tt],
                                          in_=krot2[0:64, tt])

                # B: q projection (4 nope chunks + 4 per-head rope chunks)
                for fc in range(4):
                    qp = ps_proj.tile([128, 512], F32, tag="proj")
                    nc.tensor.matmul(qp[:], sb_wqb[:, fc], qa_n[:, tt],
                                     start=True, stop=True)
                    nc.scalar.copy(q_nope[:, fc, tt], qp[:])
                for pair in range(2):
                    qp = ps_proj.tile([128, 512], F32, tag="proj")
                    nc.tensor.matmul(qp[:], sb_wqbr[:, pair], qa_n[:, tt],
                                     start=True, stop=True)
                    qpr = ps_proj.tile([128, 512], F32, tag="proj")
                    nc.tensor.matmul(qpr[:], sb_wqbr[:, 2 + pair], qa_n[:, tt],
                                     start=True, stop=True)
                    t1 = ropep.tile([128, 512], BF16, tag="qr_t1")
                    u = ropep.tile([128, 512], BF16, tag="qr_u")
                    nc.vector.tensor_mul(t1[:], qp[:], sb_cos2[:, tt])
                    nc.vector.tensor_mul(u[:], qpr[:], sb_sin2[:, tt])
                    nc.vector.tensor_add(q_rope[:, pair, tt], t1[:], u[:])
                # k_nope per head
                for h in range(HPC):
                    kp = ps_proj.tile([128, 512], F32, tag="proj")
                    nc.tensor.matmul(kp[:], sb_wk[:, h], kv_n[:, tt],
                                     start=True, stop=True)
                    nc.scalar.copy(k_nope[:, h, tt], kp[:])
                # v token-major per 128-token block
                for tb in range(4):
                    tkb = 4 * j + tb
                    vp = ps_proj.tile([128, 512], F32, tag="proj")
                    nc.tensor.matmul(
                        vp[:], kv_n[:, 128 * tkb:128 * tkb + 128], sb_wv[:],
                        start=True, stop=True)
                    nc.vector.tensor_copy(out=v_tok[:, tkb, :], in_=vp[:])

        # ---------------- Phase C+D: attention + output projection ------
        with ExitStack() as cctx:
            cpool = cctx.enter_context(tc.tile_pool(name="cpool", bufs=1))
            ep = cctx.enter_context(tc.tile_pool(name="ep", bufs=6))
            dstat = cctx.enter_context(tc.tile_pool(name="dstat", bufs=3))
            ostg = cctx.enter_context(tc.tile_pool(name="ostg", bufs=4))
            ps_s = cctx.enter_context(
                tc.tile_pool(name="ps_s", bufs=2, space="PSUM"))
            ps_av = cctx.enter_context(
                tc.tile_pool(name="ps_av", bufs=2, space="PSUM"))
            ps_den = cctx.enter_context(
                tc.tile_pool(name="ps_den", bufs=2, space="PSUM"))
            ps_o = cctx.enter_context(
                tc.tile_pool(name="ps_o", bufs=2, space="PSUM"))

            av_n = cpool.tile([128, HPC, S], BF16)

            for j in range(NJ):
                for h in range(HPC):
                    av = ps_av.tile([128, 512], F32, tag="av")
                    den = ps_den.tile([1, 512], F32, tag="den")
                    nb = 4 * j + 4          # causal: key blocks 0..4j+3
                    for i in range(nb):
                        lo = max(512 * j, 128 * i)
                        w = 512 * j + 512 - lo
                        co = lo - 512 * j
                        qs = slice(lo, 512 * j + 512)
                        sp = ps_s.tile([128, 512], F32, tag="s")
                        nc.tensor.matmul(
                            sp[:, :w],
                            k_nope[:, h, 128 * i:128 * i + 128],
                            q_nope[:, h, qs], start=True, stop=False)
                        pair, g = h // 2, 64 * (h % 2)
                        nc.tensor.matmul(
                            sp[:, :w],
                            krot2[g:g + 64, 128 * i:128 * i + 128],
                            q_rope[g:g + 64, pair, qs],
                            start=False, stop=True)
                        et = ep.tile([128, 512], BF16, tag="e")
                        nc.scalar.activation(et[:, :w], sp[:, :w], AF.Exp)
                        if i >= 4 * j:
                            nc.vector.tensor_mul(et[:, 0:128], et[:, 0:128],
                                                 sb_tril[:])
                        nc.tensor.matmul(av[:, co:512],
                                         v_tok[:, i, 128 * h:128 * h + 128],
                                         et[:, :w],
                                         start=(i == 0), stop=(i == nb - 1))
                        nc.tensor.matmul(den[:, co:512], sb_ones[:],
                                         et[:, :w],
                                         start=(i == 0), stop=(i == nb - 1))
                    rd0 = dstat.tile([1, 512], F32, tag="rd0")
                    nc.vector.reciprocal(rd0[:], den[:])
                    rd = dstat.tile([128, 512], F32, tag="rd")
                    nc.gpsimd.partition_broadcast(rd[:], rd0[:])
                    nc.vector.tensor_mul(av_n[:, h, 512 * j:512 * j + 512],
                                         av[:], rd[:])
                # D: output projection for this token tile
                for tb in range(4):
                    tkb = 4 * j + tb
                    for dt in range(4):
                        op = ps_o.tile([128, 512], F32, tag="o")
                        for h in range(HPC):
                            nc.tensor.matmul(
                                op[:],
                                av_n[:, h, 128 * tkb:128 * tkb + 128],
                                sb_wo[:, h, 512 * dt:512 * dt + 512],
                                start=(h == 0), stop=(h == HPC - 1))
                        ot = ostg.tile([128, 512], F32, tag="ot")
                        if dt % 2 == 0:
                            nc.scalar.copy(ot[:], op[:])
                        else:
                            nc.vector.tensor_copy(out=ot[:], in_=op[:])
                        nc.sync.dma_start(
                            out=out_p[128 * tkb:128 * tkb + 128,
                                      512 * dt:512 * dt + 512],
                            in_=ot[:])

    nc.compile()
    _CACHE["nc"] = nc
    return nc


def _prep_inputs(x, cos, sin, mla_norm_w, q_a_norm_w, kv_a_norm_w,
                 Wqa, Wqb, Wkva, Wkvb, Wo):
    """Host-side sharding: slice/fold/transpose weights, cast to bf16."""
    bf = ml_dtypes.bfloat16
    f32 = np.float32
    x = np.asarray(x, f32)
    cos = np.asarray(cos, f32)
    sin = np.asarray(sin, f32)
    mla_norm_w = np.asarray(mla_norm_w, f32)
    q_a_norm_w = np.asarray(q_a_norm_w, f32)
    kv_a_norm_w = np.asarray(kv_a_norm_w, f32)
    Wqa = np.asarray(Wqa, f32)
    Wqb = np.asarray(Wqb, f32)
    Wkva = np.asarray(Wkva, f32)
    Wkvb = np.asarray(Wkvb, f32)
    Wo = np.asarray(Wo, f32)

    def rot_cols(w):
        # rot(x)[f] = -x[f+32] for f<32 else x[f-32], so rot(x) = x @ rot_cols(W)
        return np.concatenate([-w[:, 32:64], w[:, 0:32]], axis=1)

    wlat_full = (np.concatenate([Wqa, Wkva], axis=1)
                 * mla_norm_w[:, None])                      # [D, 320]
    wlat_full = np.concatenate(
        [wlat_full, rot_cols(wlat_full[:, 256:320])], axis=1)  # [D, 384]
    wlat_np = np.ascontiguousarray(
        wlat_full.reshape(KCH, 128, 384).transpose(1, 0, 2)).astype(bf)

    wqb_full = (Wqb * q_a_norm_w[:, None]
                / math.sqrt(DQK)).reshape(RQ, NH, DQK)       # [128, 16, 192]
    wkvb_full = (Wkvb * kv_a_norm_w[:, None]).reshape(RKV, NH, DN + DV)

    cosT = np.ascontiguousarray(cos[0, :, 0, :].T)           # [64, S]
    sinT = np.ascontiguousarray(sin[0, :, 0, :].T)
    cos2_np = np.concatenate([cosT, cosT], axis=0).astype(bf)
    sin2_np = np.concatenate([sinT, sinT], axis=0).astype(bf)
    trilm = np.triu(np.ones((128, 128), f32)).astype(bf)     # keep kt <= qt
    kvw_np = np.ascontiguousarray(kv_a_norm_w[:, None]).astype(f32)

    in_maps = []
    for c in range(NCORES):
        b, g = divmod(c, HPC)
        hs = slice(HPC * g, HPC * g + HPC)
        xT_np = np.ascontiguousarray(
            x[b].T.reshape(KCH, 128, S).transpose(1, 0, 2)).astype(bf)
        wqb_np = np.ascontiguousarray(wqb_full[:, hs, :DN]).astype(bf)
        ropeW = wqb_full[:, hs, DN:]                         # [128, 4, 64]
        ropeWr = np.stack([rot_cols(ropeW[:, i]) for i in range(HPC)], axis=1)
        wqbr_np = np.ascontiguousarray(np.concatenate(
            [ropeW.reshape(RQ, 2, 128), ropeWr.reshape(RQ, 2, 128)],
            axis=1)).astype(bf)
        wk_np = np.ascontiguousarray(wkvb_full[:, hs, :DN]).astype(bf)
        wv_np = np.ascontiguousarray(
            wkvb_full[:, hs, DN:].reshape(RKV, HPC * DV)).astype(bf)
        wo_np = np.ascontiguousarray(
            Wo.reshape(NH, DV, D)[hs].transpose(1, 0, 2)).astype(bf)
        in_maps.append({
            "xT": xT_np, "wlat": wlat_np, "wqb": wqb_np, "wqbr": wqbr_np,
            "wk": wk_np, "wv": wv_np, "wo": wo_np,
            "cos2": cos2_np, "sin2": sin2_np,
            "cos1": cosT.astype(f32), "sin1": sinT.astype(f32),
            "tril": trilm, "kvw": kvw_np,
        })
    return in_maps


def run(inputs, trace=False, **kw):
    nc = _build()
    in_maps = _prep_inputs(**inputs)
    res = bass_utils.run_bass_kernel_spmd(
        nc, in_maps, core_ids=list(range(NCORES)), trace=trace, **kw)
    out = np.zeros((B, S, D), np.float32)
    kvp = np.zeros((B, S, RKV), np.float32)
    krot = np.zeros((B, S, DR), np.float32)
    for c in range(NCORES):
        b = c // HPC
        out[b] += res.results[c]["out_p"]
    for b in range(B):
        kvp[b] = res.results[HPC * b]["kvp_T"].T
        krot[b] = res.results[HPC * b]["krot_T"].T
    return (out, kvp, krot), res


def kernel(**inputs):
    outs, _ = run(inputs, trace=False)
    return outs


# revision 10
# speedup vs baseline: 1.0988x; 1.0988x over previous
"""DeepSeekV3 MLA Bass kernel for 8 Trainium2 NeuronCores.

Sharding: tensor-parallel across heads x data-parallel across batch.
core c: batch b = c//4, head group g = c%4 (heads 4g..4g+3).
Each core computes the shared latent path (replicated within a batch group),
attention for its 4 heads, and a partial o-projection; the host sums the 4
partial outputs per batch (TP all-reduce done host-side at gather time).

All activations live feature-major ([feature, token]) on chip so every matmul
has its contraction dim on partitions with zero on-device transposes:
  - lat = x @ [Wqa|Wkva]        -> psum [f, t], lhsT = W chunk, rhs = x^T chunk
  - scores s^T[kt, qt]          -> lhsT = k_fm block, rhs = q_fm
  - av^T[dv, qt] = v^T e^T      -> lhsT = v token-major block, rhs = e^T
  - out[t, dout]                -> lhsT = av_fm block, rhs = Wo slice
softmax denominator = ones-vector matmul over e^T (partition reduce on PE).
rmsnorm r1 of the outer norm cancels against the q/kv rmsnorms (up to eps,
~1e-6 relative); it survives only in k_rot, where it is applied explicitly
from a sum-of-squares computed via the same ones-matmul trick.
"""

import math
from contextlib import ExitStack

import numpy as np
import ml_dtypes

import concourse.bacc as bacc
import concourse.tile as tile
from concourse import mybir
from concourse import bass_utils

AF = mybir.ActivationFunctionType
ALU = mybir.AluOpType
BF16 = mybir.dt.bfloat16
F32 = mybir.dt.float32

D = 2048
NH = 16
DR = 64
DN = 128
DV = 128
RQ = 128
RKV = 128
DQK = DR + DN
B, S = 2, 2048
EPS = 1e-6
NCORES = 8
HPC = 4            # heads per core
NJ = S // 512      # 4 token tiles of 512
KCH = D // 128     # 16 contraction chunks over D

_CACHE = {}


def _rsqrt(nc, stats, out, ss_psum, mean_scale, eps_ap):
    """out[128,512] = 1/sqrt(ss*mean_scale + EPS), Newton-refined.

    ss_psum is [128,512] with the per-token sum replicated on every
    partition (ones-matmul with a full [128,128] ones stationary), so all
    stats ops run at full DVE/ACT lane width and no broadcast is needed."""
    m = stats.tile([128, 512], F32, tag="st_m")
    nc.scalar.activation(m[:], ss_psum, AF.Identity, bias=eps_ap,
                         scale=float(mean_scale))
    s = stats.tile([128, 512], F32, tag="st_s")
    nc.scalar.activation(s[:], m[:], AF.Sqrt)
    r0 = stats.tile([128, 512], F32, tag="st_r0")
    nc.vector.reciprocal(r0[:], s[:])
    t = stats.tile([128, 512], F32, tag="st_t")
    nc.vector.tensor_mul(t[:], r0[:], r0[:])
    nc.vector.tensor_mul(t[:], t[:], m[:])
    nc.vector.tensor_scalar(t[:], t[:], -0.5, 1.5, op0=ALU.mult, op1=ALU.add)
    nc.vector.tensor_mul(out, r0[:], t[:])


def _build():
    if "nc" in _CACHE:
        return _CACHE["nc"]

    nc = bacc.Bacc("TRN2", target_bir_lowering=False, debug=False,
                   num_devices=NCORES)

    # ---- I/O ----
    xT = nc.dram_tensor("xT", [128, KCH, S], BF16, kind="ExternalInput").ap()
    wlat = nc.dram_tensor("wlat", [128, KCH, 384], BF16, kind="ExternalInput").ap()
    wqb = nc.dram_tensor("wqb", [128, 4, 128], BF16, kind="ExternalInput").ap()
    wqbr = nc.dram_tensor("wqbr", [128, 4, 128], BF16, kind="ExternalInput").ap()
    wk = nc.dram_tensor("wk", [128, HPC, 128], BF16, kind="ExternalInput").ap()
    wv = nc.dram_tensor("wv", [128, 512], BF16, kind="ExternalInput").ap()
    wo = nc.dram_tensor("wo", [128, HPC, D], BF16, kind="ExternalInput").ap()
    cos2 = nc.dram_tensor("cos2", [128, S], BF16, kind="ExternalInput").ap()
    sin2 = nc.dram_tensor("sin2", [128, S], BF16, kind="ExternalInput").ap()
    cos1 = nc.dram_tensor("cos1", [64, S], F32, kind="ExternalInput").ap()
    sin1 = nc.dram_tensor("sin1", [64, S], F32, kind="ExternalInput").ap()
    tril = nc.dram_tensor("tril", [128, 128], BF16, kind="ExternalInput").ap()
    kvw = nc.dram_tensor("kvw", [128, 1], F32, kind="ExternalInput").ap()

    out_p = nc.dram_tensor("out_p", [S, D], F32, kind="ExternalOutput").ap()
    kvp_T = nc.dram_tensor("kvp_T", [RKV, S], F32, kind="ExternalOutput").ap()
    krot_T = nc.dram_tensor("krot_T", [DR, S], F32, kind="ExternalOutput").ap()

    with tile.TileContext(nc) as tc, ExitStack() as ctx:
        consts = ctx.enter_context(tc.tile_pool(name="consts", bufs=1))
        persist = ctx.enter_context(tc.tile_pool(name="persist", bufs=1))

        sb_wqb = consts.tile([128, 4, 128], BF16)
        sb_wqbr = consts.tile([128, 4, 128], BF16)
        sb_wk = consts.tile([128, HPC, 128], BF16)
        sb_wv = consts.tile([128, 512], BF16)
        sb_wo = consts.tile([128, HPC, D], BF16)
        sb_cos2 = consts.tile([128, S], BF16)
        sb_sin2 = consts.tile([128, S], BF16)
        sb_tril = consts.tile([128, 128], BF16)
        sb_kvw = consts.tile([128, 1], F32)
        sb_ones = consts.tile([128, 128], BF16)
        sb_eps = consts.tile([128, 1], F32)

        nc.sync.dma_start(out=sb_wqb[:], in_=wqb)
        nc.sync.dma_start(out=sb_wqbr[:], in_=wqbr)
        nc.sync.dma_start(out=sb_wk[:], in_=wk)
        nc.sync.dma_start(out=sb_wv[:], in_=wv)
        nc.sync.dma_start(out=sb_wo[:], in_=wo)
        nc.sync.dma_start(out=sb_cos2[:], in_=cos2)
        nc.sync.dma_start(out=sb_sin2[:], in_=sin2)
        nc.sync.dma_start(out=sb_tril[:], in_=tril)
        nc.sync.dma_start(out=sb_kvw[:], in_=kvw)
        nc.vector.memset(sb_ones[:], 1.0)
        nc.vector.memset(sb_eps[:], EPS)

        qa_n = persist.tile([128, S], BF16)
        kv_n = persist.tile([128, S], BF16)
        q_nope = persist.tile([128, HPC, S], BF16)
        q_rope = persist.tile([128, 2, S], BF16)  # [h_even|h_odd] x pair
        krot2 = persist.tile([128, S], BF16)      # krot stacked twice
        k_nope = persist.tile([128, HPC, S], BF16)
        v_tok = persist.tile([128, S // 128, 512], BF16)

        # ---------------- Phase A+B: latent path + projections ----------
        with ExitStack() as actx:
            apool = actx.enter_context(tc.tile_pool(name="apool", bufs=1))
            xjp = actx.enter_context(tc.tile_pool(name="xjp", bufs=2))
            sqp = actx.enter_context(tc.tile_pool(name="sqp", bufs=2))
            stats = actx.enter_context(tc.tile_pool(name="stats", bufs=1))
            rpool = actx.enter_context(tc.tile_pool(name="rpool", bufs=2))
            ropep = actx.enter_context(tc.tile_pool(name="ropep", bufs=2))
            stgp = actx.enter_context(tc.tile_pool(name="stgp", bufs=2))
            trigp = actx.enter_context(tc.tile_pool(name="trigp", bufs=2))
            ps_lat = actx.enter_context(
                tc.tile_pool(name="ps_lat", bufs=3, space="PSUM"))
            ps_ssx = actx.enter_context(
                tc.tile_pool(name="ps_ssx", bufs=1, space="PSUM"))
            ps_ssl = actx.enter_context(
                tc.tile_pool(name="ps_ssl", bufs=2, space="PSUM"))
            ps_proj = actx.enter_context(
                tc.tile_pool(name="ps_proj", bufs=2, space="PSUM"))

            sb_wlat = apool.tile([128, KCH, 384], BF16)
            nc.sync.dma_start(out=sb_wlat[:], in_=wlat)

            for j in range(NJ):
                tt = slice(512 * j, 512 * j + 512)
                xj = xjp.tile([128, KCH, 512], BF16, tag="xj")
                nc.sync.dma_start(out=xj[:], in_=xT[:, :, tt])
                c1 = trigp.tile([64, 512], F32, tag="c1")
                s1 = trigp.tile([64, 512], F32, tag="s1")
                nc.sync.dma_start(out=c1[:], in_=cos1[:, tt])
                nc.sync.dma_start(out=s1[:], in_=sin1[:, tt])

                # sum of squares of x per token -> r1 (used only by k_rot)
                ssx = ps_ssx.tile([128, 512], F32, tag="ssx")
                for k in range(KCH):
                    xsq = sqp.tile([128, 512], BF16, tag="xsq")
                    nc.vector.tensor_mul(xsq[:], xj[:, k], xj[:, k])
                    nc.tensor.matmul(ssx[:], sb_ones[:], xsq[:],
                                     start=(k == 0), stop=(k == KCH - 1))
                r1 = rpool.tile([128, 512], F32, tag="r1")
                _rsqrt(nc, stats, r1[:], ssx[:], 1.0 / D, sb_eps[:, :])

                # latent projections: q_a 0:128, kv 128:256,
                # rope 256:320, rope_rot 320:384 (host-permuted copy so the
                # rotate-half needs no cross-partition DVE ops)
                lp_rope = None
                for fc, (f0, fp) in enumerate([(0, 128), (128, 128),
                                               (256, 64), (320, 64)]):
                    lp = ps_lat.tile([128, 512], F32, tag="lat")
                    for k in range(KCH):
                        nc.tensor.matmul(lp[:fp], sb_wlat[:, k, f0:f0 + fp],
                                         xj[:, k], start=(k == 0),
                                         stop=(k == KCH - 1))
                    if fc < 2:
                        # per-token rmsnorm over the 128 latent features
                        lsq = sqp.tile([128, 512], BF16, tag="lsq")
                        nc.scalar.activation(lsq[:], lp[:], AF.Square)
                        ssl = ps_ssl.tile([128, 512], F32, tag="ssl")
                        nc.tensor.matmul(ssl[:], sb_ones[:], lsq[:],
                                         start=True, stop=True)
                        rr = rpool.tile([128, 512], F32, tag="rr")
                        _rsqrt(nc, stats, rr[:], ssl[:], 1.0 / 128, sb_eps[:, :])
                        dst = qa_n if fc == 0 else kv_n
                        nc.vector.tensor_mul(dst[:, tt], lp[:], rr[:])
                        if fc == 1:
                            kvo = stgp.tile([128, 512], F32, tag="kvo")
                            nc.vector.scalar_tensor_tensor(
                                kvo[:], lp[:], sb_kvw[:], rr[:],
                                op0=ALU.mult, op1=ALU.mult)
                            nc.sync.dma_start(out=kvp_T[:, tt], in_=kvo[:])
                    elif fc == 2:
                        lp_rope = lp
                    else:
                        # krot = raw*cos + rot*sin, then * r1
                        kro = stgp.tile([64, 512], F32, tag="kro")
                        t1 = ropep.tile([64, 512], F32, tag="kr_t1")
                        u = ropep.tile([64, 512], F32, tag="kr_u")
                        nc.vector.tensor_mul(t1[:], lp_rope[0:64], c1[:])
                        nc.vector.tensor_mul(u[:], lp[0:64], s1[:])
                        nc.vector.tensor_add(kro[:], t1[:], u[:])
                        nc.vector.tensor_mul(kro[:], kro[:], r1[0:64])
                        nc.sync.dma_start(out=krot_T[:, tt], in_=kro[:])
                        nc.vector.tensor_copy(out=krot2[0:64, tt], in_=kro[:])
                        nc.sync.dma_start(out=krot2[64:128, tt],
                                          in_=krot2[0:64, tt])

                # B: q projection (4 nope chunks + 4 per-head rope chunks)
                for fc in range(4):
                    qp = ps_proj.tile([128, 512], F32, tag="proj")
                    nc.tensor.matmul(qp[:], sb_wqb[:, fc], qa_n[:, tt],
                                     start=True, stop=True)
                    nc.scalar.copy(q_nope[:, fc, tt], qp[:])
                for pair in range(2):
                    qp = ps_proj.tile([128, 512], F32, tag="proj")
                    nc.tensor.matmul(qp[:], sb_wqbr[:, pair], qa_n[:, tt],
                                     start=True, stop=True)
                    qpr = ps_proj.tile([128, 512], F32, tag="proj")
                    nc.tensor.matmul(qpr[:], sb_wqbr[:, 2 + pair], qa_n[:, tt],
                                     start=True, stop=True)
                    t1 = ropep.tile([128, 512], BF16, tag="qr_t1")
                    u = ropep.tile([128, 512], BF16, tag="qr_u")
                    nc.vector.tensor_mul(t1[:], qp[:], sb_cos2[:, tt])
                    nc.vector.tensor_mul(u[:], qpr[:], sb_sin2[:, tt])
                    nc.vector.tensor_add(q_rope[:, pair, tt], t1[:], u[:])
                # k_nope per head
                for h in range(HPC):
                    kp = ps_proj.tile([128, 512], F32, tag="proj")
                    nc.tensor.matmul(kp[:], sb_wk[:, h], kv_n[:, tt],
                                     start=True, stop=True)
                    nc.scalar.copy(k_nope[:, h, tt], kp[:])
                # v token-major per 128-token block
                for tb in range(4):
                    tkb = 4 * j + tb
                    vp = ps_proj.tile([128, 512], F32, tag="proj")
                    nc.tensor.matmul(
                        vp[:], kv_n[:, 128 * tkb:128 * tkb + 128], sb_wv[:],
                        start=True, stop=True)
                    nc.vector.tensor_copy(out=v_tok[:, tkb, :], in_=vp[:])

        # ---------------- Phase C+D: attention + output projection ------
        with ExitStack() as cctx:
            cpool = cctx.enter_context(tc.tile_pool(name="cpool", bufs=1))
            ep = cctx.enter_context(tc.tile_pool(name="ep", bufs=8))
            dstat = cctx.enter_context(tc.tile_pool(name="dstat", bufs=3))
            ostg = cctx.enter_context(tc.tile_pool(name="ostg", bufs=4))
            ps_s = cctx.enter_context(
                tc.tile_pool(name="ps_s", bufs=3, space="PSUM"))
            ps_av = cctx.enter_context(
                tc.tile_pool(name="ps_av", bufs=2, space="PSUM"))
            ps_den = cctx.enter_context(
                tc.tile_pool(name="ps_den", bufs=1, space="PSUM"))
            ps_o = cctx.enter_context(
                tc.tile_pool(name="ps_o", bufs=2, space="PSUM"))

            av_n = cpool.tile([128, HPC, S], BF16)

            for j in range(NJ):
                for h in range(HPC):
                    av = ps_av.tile([128, 512], F32, tag="av")
                    den = ps_den.tile([128, 512], F32, tag="den")
                    nb = 4 * j + 4          # causal: key blocks 0..4j+3
                    for i in range(nb):
                        lo = max(512 * j, 128 * i)
                        w = 512 * j + 512 - lo
                        co = lo - 512 * j
                        qs = slice(lo, 512 * j + 512)
                        sp = ps_s.tile([128, 512], F32, tag="s")
                        nc.tensor.matmul(
                            sp[:, :w],
                            k_nope[:, h, 128 * i:128 * i + 128],
                            q_nope[:, h, qs], start=True, stop=False)
                        pair, g = h // 2, 64 * (h % 2)
                        nc.tensor.matmul(
                            sp[:, :w],
                            krot2[g:g + 64, 128 * i:128 * i + 128],
                            q_rope[g:g + 64, pair, qs],
                            start=False, stop=True)
                        et = ep.tile([128, 512], BF16, tag="e")
                        nc.scalar.activation(et[:, :w], sp[:, :w], AF.Exp)
                        if i >= 4 * j:
                            nc.vector.tensor_mul(et[:, 0:128], et[:, 0:128],
                                                 sb_tril[:])
                        nc.tensor.matmul(av[:, co:512],
                                         v_tok[:, i, 128 * h:128 * h + 128],
                                         et[:, :w],
                                         start=(i == 0), stop=(i == nb - 1))
                        nc.tensor.matmul(den[:, co:512], sb_ones[:],
                                         et[:, :w],
                                         start=(i == 0), stop=(i == nb - 1))
                    rd = dstat.tile([128, 512], F32, tag="rd")
                    nc.vector.reciprocal(rd[:], den[:])
                    nc.vector.tensor_mul(av_n[:, h, 512 * j:512 * j + 512],
                                         av[:], rd[:])
                # D: output projection for this token tile
                for tb in range(4):
                    tkb = 4 * j + tb
                    for dt in range(4):
                        op = ps_o.tile([128, 512], F32, tag="o")
                        for h in range(HPC):
                            nc.tensor.matmul(
                                op[:],
                                av_n[:, h, 128 * tkb:128 * tkb + 128],
                                sb_wo[:, h, 512 * dt:512 * dt + 512],
                                start=(h == 0), stop=(h == HPC - 1))
                        ot = ostg.tile([128, 512], F32, tag="ot")
                        if dt % 2 == 0:
                            nc.scalar.copy(ot[:], op[:])
                        else:
                            nc.vector.tensor_copy(out=ot[:], in_=op[:])
                        nc.sync.dma_start(
                            out=out_p[128 * tkb:128 * tkb + 128,
                                      512 * dt:512 * dt + 512],
                            in_=ot[:])

    nc.compile()
    _CACHE["nc"] = nc
    return nc


def _prep_inputs(x, cos, sin, mla_norm_w, q_a_norm_w, kv_a_norm_w,
                 Wqa, Wqb, Wkva, Wkvb, Wo):
    """Host-side sharding: slice/fold/transpose weights, cast to bf16."""
    bf = ml_dtypes.bfloat16
    f32 = np.float32
    x = np.asarray(x, f32)
    cos = np.asarray(cos, f32)
    sin = np.asarray(sin, f32)
    mla_norm_w = np.asarray(mla_norm_w, f32)
    q_a_norm_w = np.asarray(q_a_norm_w, f32)
    kv_a_norm_w = np.asarray(kv_a_norm_w, f32)
    Wqa = np.asarray(Wqa, f32)
    Wqb = np.asarray(Wqb, f32)
    Wkva = np.asarray(Wkva, f32)
    Wkvb = np.asarray(Wkvb, f32)
    Wo = np.asarray(Wo, f32)

    def rot_cols(w):
        # rot(x)[f] = -x[f+32] for f<32 else x[f-32], so rot(x) = x @ rot_cols(W)
        return np.concatenate([-w[:, 32:64], w[:, 0:32]], axis=1)

    wlat_full = (np.concatenate([Wqa, Wkva], axis=1)
                 * mla_norm_w[:, None])                      # [D, 320]
    wlat_full = np.concatenate(
        [wlat_full, rot_cols(wlat_full[:, 256:320])], axis=1)  # [D, 384]
    wlat_np = np.ascontiguousarray(
        wlat_full.reshape(KCH, 128, 384).transpose(1, 0, 2)).astype(bf)

    wqb_full = (Wqb * q_a_norm_w[:, None]
                / math.sqrt(DQK)).reshape(RQ, NH, DQK)       # [128, 16, 192]
    wkvb_full = (Wkvb * kv_a_norm_w[:, None]).reshape(RKV, NH, DN + DV)

    cosT = np.ascontiguousarray(cos[0, :, 0, :].T)           # [64, S]
    sinT = np.ascontiguousarray(sin[0, :, 0, :].T)
    cos2_np = np.concatenate([cosT, cosT], axis=0).astype(bf)
    sin2_np = np.concatenate([sinT, sinT], axis=0).astype(bf)
    trilm = np.triu(np.ones((128, 128), f32)).astype(bf)     # keep kt <= qt
    kvw_np = np.ascontiguousarray(kv_a_norm_w[:, None]).astype(f32)

    in_maps = []
    for c in range(NCORES):
        b, g = divmod(c, HPC)
        hs = slice(HPC * g, HPC * g + HPC)
        xT_np = np.ascontiguousarray(
            x[b].T.reshape(KCH, 128, S).transpose(1, 0, 2)).astype(bf)
        wqb_np = np.ascontiguousarray(wqb_full[:, hs, :DN]).astype(bf)
        ropeW = wqb_full[:, hs, DN:]                         # [128, 4, 64]
        ropeWr = np.stack([rot_cols(ropeW[:, i]) for i in range(HPC)], axis=1)
        wqbr_np = np.ascontiguousarray(np.concatenate(
            [ropeW.reshape(RQ, 2, 128), ropeWr.reshape(RQ, 2, 128)],
            axis=1)).astype(bf)
        wk_np = np.ascontiguousarray(wkvb_full[:, hs, :DN]).astype(bf)
        wv_np = np.ascontiguousarray(
            wkvb_full[:, hs, DN:].reshape(RKV, HPC * DV)).astype(bf)
        wo_np = np.ascontiguousarray(
            Wo.reshape(NH, DV, D)[hs].transpose(1, 0, 2)).astype(bf)
        in_maps.append({
            "xT": xT_np, "wlat": wlat_np, "wqb": wqb_np, "wqbr": wqbr_np,
            "wk": wk_np, "wv": wv_np, "wo": wo_np,
            "cos2": cos2_np, "sin2": sin2_np,
            "cos1": cosT.astype(f32), "sin1": sinT.astype(f32),
            "tril": trilm, "kvw": kvw_np,
        })
    return in_maps


def run(inputs, trace=False, **kw):
    nc = _build()
    in_maps = _prep_inputs(**inputs)
    res = bass_utils.run_bass_kernel_spmd(
        nc, in_maps, core_ids=list(range(NCORES)), trace=trace, **kw)
    out = np.zeros((B, S, D), np.float32)
    kvp = np.zeros((B, S, RKV), np.float32)
    krot = np.zeros((B, S, DR), np.float32)
    for c in range(NCORES):
        b = c // HPC
        out[b] += res.results[c]["out_p"]
    for b in range(B):
        kvp[b] = res.results[HPC * b]["kvp_T"].T
        krot[b] = res.results[HPC * b]["krot_T"].T
    return (out, kvp, krot), res


def kernel(**inputs):
    outs, _ = run(inputs, trace=False)
    return outs
